# revision 12
# baseline (speedup 1.0000x reference)
"""Trainium2 Bass kernel for nn_NormalizedDelinear (whitened linear layer).

Math (reference):
    X = x.reshape(-1, 512); N = X.shape[0]
    mean = X.mean(0);  cov = eps*I + (X-mean)^T (X-mean) / N
    C = newton_schulz_isqrt(cov, 5)
    w = weight.reshape(-1, 512) @ C;  b = bias - (w @ mean).reshape(1024, 2).sum(1)
    out = x @ w.reshape(1024, 1024).T + b

Distribution: data-parallel over the 65536 rows of x across 8 NeuronCores.
Each core computes partial S = X_loc^T X_loc and column sums s, a single
~1 MB AllReduce combines them, every core runs the (cheap, replicated)
Newton-Schulz and weight transform, then computes its slice of the output
GEMM. x is loaded from HBM once, cast f32->bf16 in-flight by the DMA
engines, and stays resident in SBUF for the second (GEMM) pass; the x^T
tiles the GEMM needs are produced by DMA xbar transposes out of that
resident copy.
"""
import numpy as np

import concourse.bacc as bacc
import concourse.mybir as mybir
import concourse.tile as tile
import concourse.bass_utils as bass_utils

N_CORES = 8
ROWS = 65536
D = 1024
BLOCK = 512
EPS = 1e-5
N_ITER = 5
PART = 128
ROWS_PER_CORE = ROWS // N_CORES  # 8192
N_ROW_TILES = ROWS_PER_CORE // PART  # 64
TILES_PER_CHUNK = 4

f32 = mybir.dt.float32
bf16 = mybir.dt.bfloat16
ADD = mybir.AluOpType.add
MUL = mybir.AluOpType.mult

import os

# Use PE (tensor engine) transposes in pass D instead of DMA xbar transposes.
USE_PE_TRANSPOSE_D = os.environ.get("NDL_PE_T", "0") == "1"
# Stage the weight bf16 cast through SBUF instead of a DRAM->DRAM cast DMA.
W_VIA_SBUF = os.environ.get("NDL_W_SBUF", "0") == "1"
# Phase truncation for hang bisection: 1=passA, 2=+AR/A/norm, 3=+NS, 4=+wT, 5=full
PHASE = int(os.environ.get("NDL_PHASE", "5"))


def build_nc(n_row_tiles=N_ROW_TILES):
    nc = bacc.Bacc(
        "TRN2", target_bir_lowering=False, debug=False, num_devices=N_CORES
    )
    rows_pc = n_row_tiles * PART
    n_chunks = max(1, n_row_tiles // TILES_PER_CHUNK)
    tpc = n_row_tiles // n_chunks
    n_total = rows_pc * N_CORES * (D // BLOCK)  # global sample count N

    x = nc.dram_tensor("x", [rows_pc, D], f32, kind="ExternalInput")
    weight = nc.dram_tensor("weight", [D, D], f32, kind="ExternalInput")
    bias_rep = nc.dram_tensor("bias_rep", [PART, D], f32, kind="ExternalInput")
    # 1.5*I laid out as 4 partition-blocks [128, b, c]: 1.5*I[b*128+p, c]
    eye15 = nc.dram_tensor("eye15", [PART, 4, BLOCK], bf16, kind="ExternalInput")
    id_bf16 = nc.dram_tensor("id_bf16", [PART, PART], bf16, kind="ExternalInput")
    id_f32 = nc.dram_tensor("id_f32", [PART, PART], f32, kind="ExternalInput")
    out = nc.dram_tensor("out", [rows_pc, D], f32, kind="ExternalOutput")

    with tile.TileContext(nc) as tc:
        _kernel_body(
            nc, tc, x, weight, bias_rep, eye15, id_bf16, id_f32, out,
            n_row_tiles, n_chunks, tpc, n_total,
        )
    nc.compile()
    return nc


def _kernel_body(
    nc, tc, x, weight, bias_rep, eye15, id_bf16, id_f32, out,
    n_row_tiles, n_chunks, tpc, n_total,
):
    inv_n = 1.0 / float(n_total)

    # ------------- long-lived pools (left side) + DRAM -------------------
    persist = tc.alloc_tile_pool(name="persist", bufs=1, side="left")
    consts = tc.alloc_tile_pool(name="consts", bufs=1, side="left")
    dram = tc.alloc_tile_pool(name="dram", bufs=1, space="DRAM")

    # resident bf16 copy of this core's x shard, chunked for dep granularity
    slab = [
        persist.tile([PART, tpc, D], bf16, tag=f"slab{c}", name=f"slab{c}")
        for c in range(n_chunks)
    ]
    eye15_sb = consts.tile([PART, 4, BLOCK], bf16, tag="eye15")
    id_bf = consts.tile([PART, PART], bf16, tag="id_bf")
    id_f = consts.tile([PART, PART], f32, tag="id_f")
    ones_f = consts.tile([PART, 1], f32, tag="ones_f")
    ones_row = consts.tile([1, PART], f32, tag="ones_row")
    ones_bf = consts.tile([PART, PART], bf16, tag="ones_bf")

    nc.sync.dma_start(eye15_sb[:], eye15[:])
    nc.sync.dma_start(id_bf[:], id_bf16[:])
    nc.sync.dma_start(id_f[:], id_f32[:])
    nc.vector.memset(ones_f[:], 1.0)
    nc.vector.memset(ones_row[:], 1.0)
    nc.vector.memset(ones_bf[:], 1.0)

    # weight cast to bf16, staged in DRAM for later transposed reads
    wbf = dram.tile([D, D], bf16, tag="wbf")
    if W_VIA_SBUF:
        wstage = tc.alloc_tile_pool(name="wstage", bufs=2, side="right")
        for q in range(4):
            wst = wstage.tile([PART, 2, D], bf16, tag="wst", name="wst")
            nc.gpsimd.dma_start(
                wst[:],
                weight[q * 2 * PART:(q + 1) * 2 * PART, :].rearrange(
                    "(t p) f -> p t f", p=PART
                ),
            )
            nc.sync.dma_start(
                wbf[q * 2 * PART:(q + 1) * 2 * PART, :].rearrange(
                    "(t p) f -> p t f", p=PART
                ),
                wst[:],
            )
        wstage.release()
    else:
        nc.gpsimd.dma_start(wbf[:], weight[:])  # DRAM -> DRAM, f32 -> bf16

    # ------------- pass A: load x (cast bf16) + S = X^T X + col sums -----
    pa = tc.alloc_tile_pool(name="passA", bufs=1, side="right")
    acc = pa.tile([PART, BLOCK], f32, tag="acc")  # running column sums
    nc.vector.memset(acc[:], 0.0)

    ps_S = tc.alloc_tile_pool(name="psumS", bufs=1, space="PSUM", side="right")
    # upper-triangle blocks of S: block row m covers columns [m*128, 512)
    s_psum = [
        ps_S.tile([PART, BLOCK - m * PART], f32, tag=f"S{m}", name=f"S{m}")
        for m in range(4)
    ]

    for c in range(n_chunks):
        src = x[c * tpc * PART:(c + 1) * tpc * PART, :].rearrange(
            "(t p) f -> p t f", p=PART
        )
        nc.gpsimd.dma_start(slab[c][:], src)  # f32 -> bf16 cast in DMA

    first = True
    for c in range(n_chunks):
        for t in range(tpc):
            for h in range(2):
                xt = slab[c][:, t, h * BLOCK:(h + 1) * BLOCK]  # [128, 512] bf16
                for m in range(4):
                    nc.tensor.matmul(
                        s_psum[m][:],
                        xt[:, m * PART:(m + 1) * PART],
                        xt[:, m * PART:],
                        start=first,
                        stop=(c == n_chunks - 1 and t == tpc - 1 and h == 1),
                    )
                # column-sum accumulator on DVE (f32 += bf16)
                nc.vector.tensor_add(acc[:], acc[:], xt)
                first = False

    # ------------- assemble full S in SBUF + pack AllReduce buffer --------
    early = tc.alloc_tile_pool(name="early", bufs=1, side="left")
    late = tc.alloc_tile_pool(name="late", bufs=1, side="right")
    ps_asm = tc.alloc_tile_pool(name="psumA", bufs=2, space="PSUM", side="left")

    S_sb = early.tile([PART, 4, BLOCK], f32, tag="S_sb")  # also reused as A
    for m in range(4):
        nc.vector.tensor_copy(S_sb[:, m, m * PART:], s_psum[m][:])
    # lower triangle: block (m, b) with b < m = transpose of block (b, m)
    for m in range(4):
        for b in range(m):
            tp = ps_asm.tile([PART, BLOCK], f32, tag="t", name="tp")
            nc.tensor.transpose(
                tp[:, 0:PART], S_sb[:, b, m * PART:(m + 1) * PART], id_f[:]
            )
            nc.vector.tensor_copy(S_sb[:, m, b * PART:(b + 1) * PART], tp[:, 0:PART])

    ps_S.release()

    if PHASE <= 1:
        nc.sync.dma_start(
            out[0:BLOCK, 0:BLOCK].rearrange("(b p) c -> p b c", p=PART), S_sb[:]
        )
        for pool in (ps_asm, late, early, pa, consts, persist, dram):
            pool.release()
        return

    # Flat partition-major AllReduce buffer: [:, 0:2048] = S (native SBUF
    # layout), [:, 2048:2560] = per-partition column-sum partials.
    AR_W = 4 * BLOCK + BLOCK
    ar_in = dram.tile([PART, AR_W], f32, tag="ar_in")
    ar_out = dram.tile([PART, AR_W], f32, tag="ar_out")
    nc.sync.dma_start(ar_in[:, 0:4 * BLOCK], S_sb[:])
    nc.sync.dma_start(ar_in[:, 4 * BLOCK:], acc[:])
    nc.gpsimd.collective_compute(
        "AllReduce",
        ADD,
        replica_groups=[list(range(N_CORES))],
        ins=[ar_in.opt()],
        outs=[ar_out.opt()],
    )

    # ------------- unpack AllReduce, build A = cov (in place of S_sb) -----
    nc.sync.dma_start(S_sb[:], ar_out[:, 0:4 * BLOCK])
    nc.sync.dma_start(acc[:], ar_out[:, 4 * BLOCK:])  # now global partials

    # global column sums s = ones^T @ acc -> [1, 512]
    scol = ps_asm.tile([PART, BLOCK], f32, tag="t")
    nc.tensor.matmul(scol[0:1, :], ones_f[:], acc[:])
    s_sb = late.tile([1, BLOCK], f32, tag="s_sb")
    nc.vector.tensor_copy(s_sb[:], scol[0:1, :])

    # meanrow[p, c] = mean[c] (replicated down partitions), via PE ones
    mr_ps = ps_asm.tile([PART, BLOCK], f32, tag="t")
    nc.tensor.matmul(mr_ps[:], ones_row[:], s_sb[:])
    meanrow = early.tile([PART, BLOCK], f32, tag="meanrow")
    nc.vector.tensor_scalar_mul(meanrow[:], mr_ps[:], inv_n)

    # mean_sb[p, b] = mean[b*128+p], via PE transposes of meanrow blocks
    mean_sb = late.tile([PART, 4], f32, tag="mean_sb")
    for b in range(4):
        mt = ps_asm.tile([PART, BLOCK], f32, tag="t", name="mt")
        nc.tensor.transpose(
            mt[:, 0:PART], meanrow[:, b * PART:(b + 1) * PART], id_f[:]
        )
        nc.vector.tensor_copy(mean_sb[:, b:b + 1], mt[:, 0:1])

    A = S_sb  # A built in place over the all-reduced S
    scratch = early.tile([PART, BLOCK], f32, tag="scratch")
    for b in range(4):
        nc.vector.tensor_scalar(
            scratch[:], meanrow[:], mean_sb[:, b:b + 1], None, op0=MUL
        )
        nc.vector.tensor_scalar_mul(A[:, b, :], A[:, b, :], inv_n)
        nc.vector.tensor_sub(A[:, b, :], A[:, b, :], scratch[:])
        # + eps*I, derived from the 1.5*I constant
        nc.vector.tensor_scalar_mul(scratch[:], eye15_sb[:, b, :], EPS / 1.5)
        nc.vector.tensor_add(A[:, b, :], A[:, b, :], scratch[:])

    # ------------- Frobenius norm; r = 1/||A||, q = 1/sqrt(||A||) ---------
    rowsq4 = early.tile([PART, 4], f32, tag="rowsq4")
    for b in range(4):
        nc.vector.tensor_mul(scratch[:], A[:, b, :], A[:, b, :])
        nc.vector.tensor_reduce(
            rowsq4[:, b:b + 1], scratch[:], mybir.AxisListType.X, ADD
        )
    rowsq = early.tile([PART, 1], f32, tag="rowsq")
    nc.vector.tensor_reduce(rowsq[:], rowsq4[:], mybir.AxisListType.X, ADD)
    n2_ps = ps_asm.tile([PART, BLOCK], f32, tag="t")
    nc.tensor.matmul(n2_ps[0:1, 0:1], ones_f[:], rowsq[:])
    n2_sb = early.tile([1, 1], f32, tag="n2sb")
    nc.vector.tensor_copy(n2_sb[:], n2_ps[0:1, 0:1])
    # broadcast ||A||^2 to [128, 1] then compute per-partition scalars
    n2_bc = ps_asm.tile([PART, BLOCK], f32, tag="t")
    nc.tensor.matmul(n2_bc[:, 0:1], ones_row[:], n2_sb[:])
    rq = late.tile([PART, 2], f32, tag="rq")
    nc.vector.reciprocal(rq[:, 0:1], n2_bc[:, 0:1])    # 1/||A||^2
    nc.scalar.sqrt(rq[:, 0:1], rq[:, 0:1])             # r = 1/||A||
    nc.scalar.sqrt(rq[:, 1:2], rq[:, 0:1])             # q = 1/sqrt(||A||)

    ps_asm.release()

    if PHASE <= 2:
        nc.sync.dma_start(
            out[0:BLOCK, 0:BLOCK].rearrange("(b p) c -> p b c", p=PART), A[:]
        )
        for pool in (late, early, pa, consts, persist, dram):
            pool.release()
        return

    # ------------- Newton-Schulz (bf16 matmuls, fp32 PSUM) ----------------
    wth = tc.alloc_tile_pool(name="wth", bufs=1, side="right")
    ns = tc.alloc_tile_pool(name="ns", bufs=1, side="right")
    ps_ns = tc.alloc_tile_pool(name="psumNS", bufs=6, space="PSUM", side="left")

    # ping-pong buffers for Y and Z across iterations
    Yb = [ns.tile([PART, 4, BLOCK], bf16, tag=f"Y{i}", name=f"Y{i}") for i in range(2)]
    Zb = [ns.tile([PART, 4, BLOCK], bf16, tag=f"Z{i}", name=f"Z{i}") for i in range(2)]
    T = ns.tile([PART, 4, BLOCK], bf16, tag="T")

    for b in range(4):  # Y0 = A * r
        nc.vector.tensor_scalar(Yb[0][:, b, :], A[:, b, :], rq[:, 0:1], None, op0=MUL)

    early.release()

    def mm512(dst, L, R, copy_engine, scale=None):
        """dst = L(stored)^T @ R for 512x512 bf16 operands laid [128, 4, 512].

        Valid when L is symmetric (or its transpose is wanted). dst must not
        alias L or R. copy_engine: 'v' DVE / 's' ACT for the psum->sbuf copy.
        """
        for mb in range(4):
            pt = ps_ns.tile([PART, BLOCK], f32, tag="mm", name="mm")
            for kb in range(4):
                nc.tensor.matmul(
                    pt[:],
                    L[:, kb, mb * PART:(mb + 1) * PART],
                    R[:, kb, :],
                    start=(kb == 0),
                    stop=(kb == 3),
                )
            if scale is not None:
                nc.vector.tensor_scalar(dst[:, mb, :], pt[:], scale, None, op0=MUL)
            elif copy_engine == "v":
                nc.vector.tensor_copy(dst[:, mb, :], pt[:])
            else:
                nc.scalar.copy(dst[:, mb, :], pt[:])

    def build_T(p_blocks):
        # T = 1.5 I - 0.5 * P ; p_blocks: list of 4 psum/sbuf [128, 512] APs
        for b in range(4):
            nc.vector.tensor_scalar(T[:, b, :], p_blocks[b], -0.5, None, op0=MUL)
            nc.vector.tensor_add(T[:, b, :], T[:, b, :], eye15_sb[:, b, :])

    # iter 1: Z0 = I, so P = Y0; T1 = 1.5I - 0.5 Y0; Y1 = Y0 @ T1; Z1 = T1
    Y, Z = Yb[0], Zb[0]
    build_T([Y[:, b, :] for b in range(4)])
    mm512(Yb[1], Y, T, "s")  # Y1 = Y0 @ T1  (Y0 symmetric)
    for b in range(4):
        nc.scalar.copy(Zb[1][:, b, :], T[:, b, :])
    Y, Z = Yb[1], Zb[1]

    C = None
    for it in range(1, N_ITER):
        # P = Z @ Y -> psum tiles; T = 1.5I - 0.5P
        pt_blocks = []
        for mb in range(4):
            pt = ps_ns.tile([PART, BLOCK], f32, tag="mm", name="mm")
            for kb in range(4):
                nc.tensor.matmul(
                    pt[:],
                    Z[:, kb, mb * PART:(mb + 1) * PART],
                    Y[:, kb, :],
                    start=(kb == 0),
                    stop=(kb == 3),
                )
            pt_blocks.append(pt)
        build_T([pt[:] for pt in pt_blocks])
        if it < N_ITER - 1:
            Yn, Zn = Yb[(it + 1) % 2], Zb[(it + 1) % 2]
            mm512(Yn, Y, T, "s")  # Y_next = Y @ T
            mm512(Zn, T, Z, "v")  # Z_next = T @ Z  (T symmetric)
            Y, Z = Yn, Zn
        else:
            # final iteration: only Z needed; C = q * (T @ Z).
            # At it=4, Y is Yb[1] / Z is Zb[1]; Yb[0] is dead -> reuse for C.
            C = Yb[0]
            mm512(C, T, Z, "v", scale=rq[:, 1:2])

    if PHASE <= 3:
        cdump = pa.tile([PART, 4, BLOCK], f32, tag="cdump")
        for b in range(4):
            nc.vector.tensor_copy(cdump[:, b, :], C[:, b, :])
        nc.sync.dma_start(
            out[0:BLOCK, 0:BLOCK].rearrange("(b p) c -> p b c", p=PART), cdump[:]
        )
        for pool in (ps_ns, ns, wth, late, pa, consts, persist, dram):
            pool.release()
        return

    # ------------- wT = C^T @ W^T ; b' = bias - pair-summed w @ mean -------
    wts2 = tc.alloc_tile_pool(name="wts2", bufs=1, side="left")
    wT = wts2.tile([PART, 8, D], bf16, tag="wT")  # w_full^T[i, o]
    # W^T j-half, DMA-transposed straight out of the bf16 weight in DRAM
    for j in range(2):
        WTh = wth.tile([PART, 4, D], bf16, tag="WTh", name="WTh")
        for db in range(4):
            for ob in range(8):
                nc.sync.dma_start(
                    WTh[:, db, ob * PART:(ob + 1) * PART],
                    wbf[ob * PART:(ob + 1) * PART,
                        j * BLOCK + db * PART:j * BLOCK + (db + 1) * PART],
                    transpose=True,
                )
        for cb in range(4):
            for nb in range(2):
                pt = ps_ns.tile([PART, BLOCK], f32, tag="mm", name="mm")
                for db in range(4):
                    nc.tensor.matmul(
                        pt[:],
                        C[:, db, cb * PART:(cb + 1) * PART],
                        WTh[:, db, nb * BLOCK:(nb + 1) * BLOCK],
                        start=(db == 0),
                        stop=(db == 3),
                    )
                nc.scalar.copy(
                    wT[:, j * 4 + cb, nb * BLOCK:(nb + 1) * BLOCK], pt[:]
                )

    # mean replicated blocks: rep_b[p, f] = mean[b*128+p]
    rep = [
        ns.tile([PART, PART], bf16, tag=f"rep{b}", name=f"rep{b}") for b in range(4)
    ]
    for b in range(4):
        nc.vector.tensor_scalar(
            rep[b][:], ones_bf[:], mean_sb[:, b:b + 1], None, op0=MUL
        )
    bc_ps = [
        ps_ns.tile([PART, BLOCK], f32, tag="mm", name=f"bc{i}") for i in range(2)
    ]
    for nb in range(2):
        for g in range(8):
            nc.tensor.matmul(
                bc_ps[nb][:],
                rep[g % 4][:],
                wT[:, g, nb * BLOCK:(nb + 1) * BLOCK],
                start=(g == 0),
                stop=(g == 7),
            )
    b_rep = wts2.tile([PART, D], f32, tag="b_rep")  # b' replicated on partitions
    nc.sync.dma_start(b_rep[:], bias_rep[:])
    for nb in range(2):
        nc.vector.tensor_sub(
            b_rep[:, nb * BLOCK:(nb + 1) * BLOCK],
            b_rep[:, nb * BLOCK:(nb + 1) * BLOCK],
            bc_ps[nb][:],
        )

    ps_ns.release()
    ns.release()
    wth.release()
    late.release()
    pa.release()

    if PHASE <= 4:
        nc.sync.dma_start(out[0:PART, :], b_rep[:])
        for pool in (wts2, consts, persist, dram):
            pool.release()
        return

    # ------------- pass D: out = x @ w^T + b' -----------------------------
    pd = tc.alloc_tile_pool(name="passD", bufs=16, side="right")
    pd_out = tc.alloc_tile_pool(name="passDout", bufs=4, side="right")
    ps_D = tc.alloc_tile_pool(name="psumD", bufs=4, space="PSUM", side="left")
    if USE_PE_TRANSPOSE_D:
        ps_Dt = tc.alloc_tile_pool(name="psumDt", bufs=4, space="PSUM", side="left")

    for rt in range(n_row_tiles):
        c, t = divmod(rt, tpc)
        # x^T tiles for this row block: xT[g][i, n] = x[rt*128+n, g*128+i]
        xTs = []
        for g in range(8):
            xT = pd.tile([PART, PART], bf16, tag="xT", name="xT")
            src = slab[c][:, t, g * PART:(g + 1) * PART]
            if USE_PE_TRANSPOSE_D:
                tp = ps_Dt.tile([PART, PART], bf16, tag="xtr", name="xtr")
                nc.tensor.transpose(tp[:], src, id_bf[:])
                nc.scalar.copy(xT[:], tp[:])
            else:
                nc.sync.dma_start(xT[:], src, transpose=True)
            xTs.append(xT)
        for nb in range(2):
            pt = ps_D.tile([PART, BLOCK], f32, tag="outp", name="outp")
            for g in range(8):
                nc.tensor.matmul(
                    pt[:],
                    xTs[g][:],
                    wT[:, g, nb * BLOCK:(nb + 1) * BLOCK],
                    start=(g == 0),
                    stop=(g == 7),
                )
            ot = pd_out.tile([PART, BLOCK], f32, tag="ot", name="ot")
            nc.vector.tensor_add(ot[:], pt[:], b_rep[:, nb * BLOCK:(nb + 1) * BLOCK])
            nc.sync.dma_start(
                out[rt * PART:(rt + 1) * PART, nb * BLOCK:(nb + 1) * BLOCK], ot[:]
            )

    if USE_PE_TRANSPOSE_D:
        ps_Dt.release()
    ps_D.release()
    pd_out.release()
    pd.release()
    wts2.release()
    consts.release()
    persist.release()
    dram.release()


# ---------------------------------------------------------------------------
def make_aux_inputs():
    import ml_dtypes

    eye15 = np.zeros((PART, 4, BLOCK), np.float32)
    for b in range(4):
        for p in range(PART):
            eye15[p, b, b * PART + p] = 1.5
    return {
        "eye15": eye15.astype(ml_dtypes.bfloat16),
        "id_bf16": np.eye(PART, dtype=ml_dtypes.bfloat16),
        "id_f32": np.eye(PART, dtype=np.float32),
    }


_NC_CACHE = {}


def get_nc(n_row_tiles=N_ROW_TILES):
    if n_row_tiles not in _NC_CACHE:
        _NC_CACHE[n_row_tiles] = build_nc(n_row_tiles)
    return _NC_CACHE[n_row_tiles]


def make_in_maps(x, weight, bias, n_row_tiles=N_ROW_TILES):
    aux = make_aux_inputs()
    x = np.ascontiguousarray(np.asarray(x, dtype=np.float32))
    weight = np.ascontiguousarray(np.asarray(weight, dtype=np.float32))
    bias = np.asarray(bias, dtype=np.float32)
    bias_rep = np.ascontiguousarray(np.tile(bias[None, :], (PART, 1)))
    rows_pc = n_row_tiles * PART
    in_maps = []
    for i in range(N_CORES):
        m = {"x": x[i * rows_pc:(i + 1) * rows_pc], "weight": weight,
             "bias_rep": bias_rep}
        m.update(aux)
        in_maps.append(m)
    return in_maps


def kernel(x, weight, bias):
    nc = get_nc()
    in_maps = make_in_maps(x, weight, bias)
    res = bass_utils.run_bass_kernel_spmd(
        nc, in_maps, core_ids=list(range(N_CORES))
    )
    return np.concatenate([r["out"] for r in res.results], axis=0)


# revision 13
# speedup vs baseline: 2.1312x; 2.1312x over previous
"""Trainium2 Bass kernel for nn_NormalizedDelinear (whitened linear layer).

Math (reference):
    X = x.reshape(-1, 512); N = X.shape[0]
    mean = X.mean(0);  cov = eps*I + (X-mean)^T (X-mean) / N
    C = newton_schulz_isqrt(cov, 5)
    w = weight.reshape(-1, 512) @ C;  b = bias - (w @ mean).reshape(1024, 2).sum(1)
    out = x @ w.reshape(1024, 1024).T + b

Distribution: data-parallel over the 65536 rows of x across 8 NeuronCores.
Each core computes partial S = X_loc^T X_loc and column sums s, a single
~1 MB AllReduce combines them, every core runs the (cheap, replicated)
Newton-Schulz and weight transform, then computes its slice of the output
GEMM. x is loaded from HBM once, cast f32->bf16 in-flight by the DMA
engines, and stays resident in SBUF for the second (GEMM) pass; the x^T
tiles the GEMM needs are produced by DMA xbar transposes out of that
resident copy.
"""
import numpy as np

import concourse.bacc as bacc
import concourse.mybir as mybir
import concourse.tile as tile
import concourse.bass_utils as bass_utils

N_CORES = 8
ROWS = 65536
D = 1024
BLOCK = 512
EPS = 1e-5
N_ITER = 5
PART = 128
ROWS_PER_CORE = ROWS // N_CORES  # 8192
N_ROW_TILES = ROWS_PER_CORE // PART  # 64
TILES_PER_CHUNK = 4

f32 = mybir.dt.float32
bf16 = mybir.dt.bfloat16
ADD = mybir.AluOpType.add
MUL = mybir.AluOpType.mult

import os

# Use PE (tensor engine) transposes in pass D instead of DMA xbar transposes.
USE_PE_TRANSPOSE_D = os.environ.get("NDL_PE_T", "0") == "1"
# Stage the weight bf16 cast through SBUF instead of a DRAM->DRAM cast DMA.
W_VIA_SBUF = os.environ.get("NDL_W_SBUF", "0") == "1"
# Phase truncation for hang bisection: 1=passA, 2=+AR/A/norm, 3=+NS, 4=+wT, 5=full
PHASE = int(os.environ.get("NDL_PHASE", "5"))


def build_nc(n_row_tiles=N_ROW_TILES):
    nc = bacc.Bacc(
        "TRN2", target_bir_lowering=False, debug=False, num_devices=N_CORES
    )
    rows_pc = n_row_tiles * PART
    n_chunks = max(1, n_row_tiles // TILES_PER_CHUNK)
    tpc = n_row_tiles // n_chunks
    n_total = rows_pc * N_CORES * (D // BLOCK)  # global sample count N

    x = nc.dram_tensor("x", [rows_pc, D], f32, kind="ExternalInput")
    weight = nc.dram_tensor("weight", [D, D], f32, kind="ExternalInput")
    bias_rep = nc.dram_tensor("bias_rep", [PART, D], f32, kind="ExternalInput")
    # 1.5*I laid out as 4 partition-blocks [128, b, c]: 1.5*I[b*128+p, c]
    eye15 = nc.dram_tensor("eye15", [PART, 4, BLOCK], bf16, kind="ExternalInput")
    id_bf16 = nc.dram_tensor("id_bf16", [PART, PART], bf16, kind="ExternalInput")
    id_f32 = nc.dram_tensor("id_f32", [PART, PART], f32, kind="ExternalInput")
    out = nc.dram_tensor("out", [rows_pc, D], f32, kind="ExternalOutput")

    with tile.TileContext(nc) as tc:
        _kernel_body(
            nc, tc, x, weight, bias_rep, eye15, id_bf16, id_f32, out,
            n_row_tiles, n_chunks, tpc, n_total,
        )
    nc.compile()
    return nc


def _kernel_body(
    nc, tc, x, weight, bias_rep, eye15, id_bf16, id_f32, out,
    n_row_tiles, n_chunks, tpc, n_total,
):
    inv_n = 1.0 / float(n_total)

    # ------------- long-lived pools (left side) + DRAM -------------------
    persist = tc.alloc_tile_pool(name="persist", bufs=1, side="left")
    consts = tc.alloc_tile_pool(name="consts", bufs=1, side="left")
    dram = tc.alloc_tile_pool(name="dram", bufs=1, space="DRAM")

    # resident bf16 copy of this core's x shard, chunked for dep granularity
    slab = [
        persist.tile([PART, tpc, D], bf16, tag=f"slab{c}", name=f"slab{c}")
        for c in range(n_chunks)
    ]
    eye15_sb = consts.tile([PART, 4, BLOCK], bf16, tag="eye15")
    id_bf = consts.tile([PART, PART], bf16, tag="id_bf")
    id_f = consts.tile([PART, PART], f32, tag="id_f")
    ones_f = consts.tile([PART, 1], f32, tag="ones_f")
    ones_row = consts.tile([1, PART], f32, tag="ones_row")
    ones_bf = consts.tile([PART, PART], bf16, tag="ones_bf")

    nc.sync.dma_start(eye15_sb[:], eye15[:])
    nc.sync.dma_start(id_bf[:], id_bf16[:])
    nc.sync.dma_start(id_f[:], id_f32[:])
    nc.vector.memset(ones_f[:], 1.0)
    nc.vector.memset(ones_row[:], 1.0)
    nc.vector.memset(ones_bf[:], 1.0)

    # weight cast to bf16, staged in DRAM for later transposed reads
    wbf = dram.tile([D, D], bf16, tag="wbf")
    if W_VIA_SBUF:
        wstage = tc.alloc_tile_pool(name="wstage", bufs=2, side="right")
        for q in range(4):
            wst = wstage.tile([PART, 2, D], bf16, tag="wst", name="wst")
            nc.gpsimd.dma_start(
                wst[:],
                weight[q * 2 * PART:(q + 1) * 2 * PART, :].rearrange(
                    "(t p) f -> p t f", p=PART
                ),
            )
            nc.sync.dma_start(
                wbf[q * 2 * PART:(q + 1) * 2 * PART, :].rearrange(
                    "(t p) f -> p t f", p=PART
                ),
                wst[:],
            )
        wstage.release()
    else:
        nc.gpsimd.dma_start(wbf[:], weight[:])  # DRAM -> DRAM, f32 -> bf16

    # ------------- pass A: load x (cast bf16) + S = X^T X + col sums -----
    pa = tc.alloc_tile_pool(name="passA", bufs=1, side="right")
    acc = pa.tile([PART, BLOCK], f32, tag="acc")  # running column sums
    nc.vector.memset(acc[:], 0.0)

    ps_S = tc.alloc_tile_pool(name="psumS", bufs=1, space="PSUM", side="right")
    # upper-triangle blocks of S: block row m covers columns [m*128, 512)
    s_psum = [
        ps_S.tile([PART, BLOCK - m * PART], f32, tag=f"S{m}", name=f"S{m}")
        for m in range(4)
    ]

    for c in range(n_chunks):
        src = x[c * tpc * PART:(c + 1) * tpc * PART, :].rearrange(
            "(t p) f -> p t f", p=PART
        )
        nc.gpsimd.dma_start(slab[c][:], src)  # f32 -> bf16 cast in DMA

    first = True
    for c in range(n_chunks):
        for t in range(tpc):
            for h in range(2):
                xt = slab[c][:, t, h * BLOCK:(h + 1) * BLOCK]  # [128, 512] bf16
                for m in range(4):
                    nc.tensor.matmul(
                        s_psum[m][:],
                        xt[:, m * PART:(m + 1) * PART],
                        xt[:, m * PART:],
                        start=first,
                        stop=(c == n_chunks - 1 and t == tpc - 1 and h == 1),
                    )
                # column-sum accumulator on DVE (f32 += bf16)
                nc.vector.tensor_add(acc[:], acc[:], xt)
                first = False

    # ------------- assemble full S in SBUF + pack AllReduce buffer --------
    early = tc.alloc_tile_pool(name="early", bufs=1, side="left")
    late = tc.alloc_tile_pool(name="late", bufs=1, side="right")
    ps_asm = tc.alloc_tile_pool(name="psumA", bufs=2, space="PSUM", side="left")

    S_sb = early.tile([PART, 4, BLOCK], f32, tag="S_sb")  # also reused as A
    for m in range(4):
        nc.vector.tensor_copy(S_sb[:, m, m * PART:], s_psum[m][:])
    # lower triangle: block (m, b) with b < m = transpose of block (b, m)
    for m in range(4):
        for b in range(m):
            tp = ps_asm.tile([PART, BLOCK], f32, tag="t", name="tp")
            nc.tensor.transpose(
                tp[:, 0:PART], S_sb[:, b, m * PART:(m + 1) * PART], id_f[:]
            )
            nc.vector.tensor_copy(S_sb[:, m, b * PART:(b + 1) * PART], tp[:, 0:PART])

    ps_S.release()

    if PHASE <= 1:
        nc.sync.dma_start(
            out[0:BLOCK, 0:BLOCK].rearrange("(b p) c -> p b c", p=PART), S_sb[:]
        )
        for pool in (ps_asm, late, early, pa, consts, persist, dram):
            pool.release()
        return

    # Flat partition-major AllReduce buffer: [:, 0:2048] = S (native SBUF
    # layout), [:, 2048:2560] = per-partition column-sum partials.
    AR_W = 4 * BLOCK + BLOCK
    ar_in = dram.tile([PART, AR_W], f32, tag="ar_in")
    ar_out = dram.tile([PART, AR_W], f32, tag="ar_out")
    nc.sync.dma_start(ar_in[:, 0:4 * BLOCK], S_sb[:])
    nc.sync.dma_start(ar_in[:, 4 * BLOCK:], acc[:])
    nc.gpsimd.collective_compute(
        "AllReduce",
        ADD,
        replica_groups=[list(range(N_CORES))],
        ins=[ar_in.opt()],
        outs=[ar_out.opt()],
    )

    # ------------- unpack AllReduce, build A = cov (in place of S_sb) -----
    nc.sync.dma_start(S_sb[:], ar_out[:, 0:4 * BLOCK])
    nc.sync.dma_start(acc[:], ar_out[:, 4 * BLOCK:])  # now global partials

    # global column sums s = ones^T @ acc -> [1, 512]
    scol = ps_asm.tile([PART, BLOCK], f32, tag="t")
    nc.tensor.matmul(scol[0:1, :], ones_f[:], acc[:])
    s_sb = late.tile([1, BLOCK], f32, tag="s_sb")
    nc.vector.tensor_copy(s_sb[:], scol[0:1, :])

    # meanrow[p, c] = mean[c] (replicated down partitions), via PE ones
    mr_ps = ps_asm.tile([PART, BLOCK], f32, tag="t")
    nc.tensor.matmul(mr_ps[:], ones_row[:], s_sb[:])
    meanrow = early.tile([PART, BLOCK], f32, tag="meanrow")
    nc.vector.tensor_scalar_mul(meanrow[:], mr_ps[:], inv_n)

    # mean_sb[p, b] = mean[b*128+p], via PE transposes of meanrow blocks
    mean_sb = late.tile([PART, 4], f32, tag="mean_sb")
    for b in range(4):
        mt = ps_asm.tile([PART, BLOCK], f32, tag="t", name="mt")
        nc.tensor.transpose(
            mt[:, 0:PART], meanrow[:, b * PART:(b + 1) * PART], id_f[:]
        )
        nc.vector.tensor_copy(mean_sb[:, b:b + 1], mt[:, 0:1])

    A = S_sb  # A built in place over the all-reduced S
    scratch = early.tile([PART, BLOCK], f32, tag="scratch")
    for b in range(4):
        nc.vector.tensor_scalar(
            scratch[:], meanrow[:], mean_sb[:, b:b + 1], None, op0=MUL
        )
        nc.vector.tensor_scalar_mul(A[:, b, :], A[:, b, :], inv_n)
        nc.vector.tensor_sub(A[:, b, :], A[:, b, :], scratch[:])
        # + eps*I, derived from the 1.5*I constant
        nc.vector.tensor_scalar_mul(scratch[:], eye15_sb[:, b, :], EPS / 1.5)
        nc.vector.tensor_add(A[:, b, :], A[:, b, :], scratch[:])

    # ------------- Frobenius norm; r = 1/||A||, q = 1/sqrt(||A||) ---------
    rowsq4 = early.tile([PART, 4], f32, tag="rowsq4")
    for b in range(4):
        nc.vector.tensor_mul(scratch[:], A[:, b, :], A[:, b, :])
        nc.vector.tensor_reduce(
            rowsq4[:, b:b + 1], scratch[:], mybir.AxisListType.X, ADD
        )
    rowsq = early.tile([PART, 1], f32, tag="rowsq")
    nc.vector.tensor_reduce(rowsq[:], rowsq4[:], mybir.AxisListType.X, ADD)
    n2_ps = ps_asm.tile([PART, BLOCK], f32, tag="t")
    nc.tensor.matmul(n2_ps[0:1, 0:1], ones_f[:], rowsq[:])
    n2_sb = early.tile([1, 1], f32, tag="n2sb")
    nc.vector.tensor_copy(n2_sb[:], n2_ps[0:1, 0:1])
    # broadcast ||A||^2 to [128, 1] then compute per-partition scalars
    n2_bc = ps_asm.tile([PART, BLOCK], f32, tag="t")
    nc.tensor.matmul(n2_bc[:, 0:1], ones_row[:], n2_sb[:])
    rq = late.tile([PART, 2], f32, tag="rq")
    nc.vector.reciprocal(rq[:, 0:1], n2_bc[:, 0:1])    # 1/||A||^2
    nc.scalar.sqrt(rq[:, 0:1], rq[:, 0:1])             # r = 1/||A||
    nc.scalar.sqrt(rq[:, 1:2], rq[:, 0:1])             # q = 1/sqrt(||A||)

    ps_asm.release()

    if PHASE <= 2:
        nc.sync.dma_start(
            out[0:BLOCK, 0:BLOCK].rearrange("(b p) c -> p b c", p=PART), A[:]
        )
        for pool in (late, early, pa, consts, persist, dram):
            pool.release()
        return

    # ------------- Newton-Schulz (bf16 matmuls, fp32 PSUM) ----------------
    wth = tc.alloc_tile_pool(name="wth", bufs=1, side="right")
    ns = tc.alloc_tile_pool(name="ns", bufs=1, side="right")
    ps_ns = tc.alloc_tile_pool(name="psumNS", bufs=6, space="PSUM", side="left")

    # ping-pong buffers for Y and Z across iterations
    Yb = [ns.tile([PART, 4, BLOCK], bf16, tag=f"Y{i}", name=f"Y{i}") for i in range(2)]
    Zb = [ns.tile([PART, 4, BLOCK], bf16, tag=f"Z{i}", name=f"Z{i}") for i in range(2)]
    T = ns.tile([PART, 4, BLOCK], bf16, tag="T")

    for b in range(4):  # Y0 = A * r
        nc.vector.tensor_scalar(Yb[0][:, b, :], A[:, b, :], rq[:, 0:1], None, op0=MUL)

    early.release()

    def mm512(dst, L, R, copy_engine, scale=None):
        """dst = L(stored)^T @ R for 512x512 bf16 operands laid [128, 4, 512].

        Valid when L is symmetric (or its transpose is wanted). dst must not
        alias L or R. copy_engine: 'v' DVE / 's' ACT for the psum->sbuf copy.
        """
        for mb in range(4):
            pt = ps_ns.tile([PART, BLOCK], f32, tag="mm", name="mm")
            for kb in range(4):
                nc.tensor.matmul(
                    pt[:],
                    L[:, kb, mb * PART:(mb + 1) * PART],
                    R[:, kb, :],
                    start=(kb == 0),
                    stop=(kb == 3),
                )
            if scale is not None:
                nc.vector.tensor_scalar(dst[:, mb, :], pt[:], scale, None, op0=MUL)
            elif copy_engine == "v":
                nc.vector.tensor_copy(dst[:, mb, :], pt[:])
            else:
                nc.scalar.copy(dst[:, mb, :], pt[:])

    def build_T(p_blocks):
        # T = 1.5 I - 0.5 * P ; p_blocks: list of 4 psum/sbuf [128, 512] APs
        for b in range(4):
            nc.vector.tensor_scalar(T[:, b, :], p_blocks[b], -0.5, None, op0=MUL)
            nc.vector.tensor_add(T[:, b, :], T[:, b, :], eye15_sb[:, b, :])

    # iter 1: Z0 = I, so P = Y0; T1 = 1.5I - 0.5 Y0; Y1 = Y0 @ T1; Z1 = T1
    Y, Z = Yb[0], Zb[0]
    build_T([Y[:, b, :] for b in range(4)])
    mm512(Yb[1], Y, T, "s")  # Y1 = Y0 @ T1  (Y0 symmetric)
    for b in range(4):
        nc.scalar.copy(Zb[1][:, b, :], T[:, b, :])
    Y, Z = Yb[1], Zb[1]

    C = None
    for it in range(1, N_ITER):
        # P = Z @ Y -> psum tiles; T = 1.5I - 0.5P
        pt_blocks = []
        for mb in range(4):
            pt = ps_ns.tile([PART, BLOCK], f32, tag="mm", name="mm")
            for kb in range(4):
                nc.tensor.matmul(
                    pt[:],
                    Z[:, kb, mb * PART:(mb + 1) * PART],
                    Y[:, kb, :],
                    start=(kb == 0),
                    stop=(kb == 3),
                )
            pt_blocks.append(pt)
        build_T([pt[:] for pt in pt_blocks])
        if it < N_ITER - 1:
            Yn, Zn = Yb[(it + 1) % 2], Zb[(it + 1) % 2]
            mm512(Yn, Y, T, "s")  # Y_next = Y @ T
            mm512(Zn, T, Z, "v")  # Z_next = T @ Z  (T symmetric)
            Y, Z = Yn, Zn
        else:
            # final iteration: only Z needed; C = q * (T @ Z).
            # At it=4, Y is Yb[1] / Z is Zb[1]; Yb[0] is dead -> reuse for C.
            C = Yb[0]
            mm512(C, T, Z, "v", scale=rq[:, 1:2])

    if PHASE <= 3:
        cdump = pa.tile([PART, 4, BLOCK], f32, tag="cdump")
        for b in range(4):
            nc.vector.tensor_copy(cdump[:, b, :], C[:, b, :])
        nc.sync.dma_start(
            out[0:BLOCK, 0:BLOCK].rearrange("(b p) c -> p b c", p=PART), cdump[:]
        )
        for pool in (ps_ns, ns, wth, late, pa, consts, persist, dram):
            pool.release()
        return

    # ------------- wT = C^T @ W^T ; b' = bias - pair-summed w @ mean -------
    wts2 = tc.alloc_tile_pool(name="wts2", bufs=1, side="left")
    wT = wts2.tile([PART, 8, D], bf16, tag="wT")  # w_full^T[i, o]
    # W^T j-half, DMA-transposed straight out of the bf16 weight in DRAM
    for j in range(2):
        WTh = wth.tile([PART, 4, D], bf16, tag="WTh", name="WTh")
        for ob in range(8):
            nc.sync.dma_start(
                WTh[:, :, ob * PART:(ob + 1) * PART],
                wbf[ob * PART:(ob + 1) * PART, j * BLOCK:(j + 1) * BLOCK],
                transpose=True,
            )
        for cb in range(4):
            for nb in range(2):
                pt = ps_ns.tile([PART, BLOCK], f32, tag="mm", name="mm")
                for db in range(4):
                    nc.tensor.matmul(
                        pt[:],
                        C[:, db, cb * PART:(cb + 1) * PART],
                        WTh[:, db, nb * BLOCK:(nb + 1) * BLOCK],
                        start=(db == 0),
                        stop=(db == 3),
                    )
                nc.scalar.copy(
                    wT[:, j * 4 + cb, nb * BLOCK:(nb + 1) * BLOCK], pt[:]
                )

    # mean replicated blocks: rep_b[p, f] = mean[b*128+p]
    rep = [
        ns.tile([PART, PART], bf16, tag=f"rep{b}", name=f"rep{b}") for b in range(4)
    ]
    for b in range(4):
        nc.vector.tensor_scalar(
            rep[b][:], ones_bf[:], mean_sb[:, b:b + 1], None, op0=MUL
        )
    bc_ps = [
        ps_ns.tile([PART, BLOCK], f32, tag="mm", name=f"bc{i}") for i in range(2)
    ]
    for nb in range(2):
        for g in range(8):
            nc.tensor.matmul(
                bc_ps[nb][:],
                rep[g % 4][:],
                wT[:, g, nb * BLOCK:(nb + 1) * BLOCK],
                start=(g == 0),
                stop=(g == 7),
            )
    b_rep = wts2.tile([PART, D], f32, tag="b_rep")  # b' replicated on partitions
    nc.sync.dma_start(b_rep[:], bias_rep[:])
    for nb in range(2):
        nc.vector.tensor_sub(
            b_rep[:, nb * BLOCK:(nb + 1) * BLOCK],
            b_rep[:, nb * BLOCK:(nb + 1) * BLOCK],
            bc_ps[nb][:],
        )

    ps_ns.release()
    ns.release()
    wth.release()
    late.release()
    pa.release()

    if PHASE <= 4:
        nc.sync.dma_start(out[0:PART, :], b_rep[:])
        for pool in (wts2, consts, persist, dram):
            pool.release()
        return

    # ------------- pass D: out = x @ w^T + b' -----------------------------
    pd = tc.alloc_tile_pool(name="passD", bufs=16, side="right")
    pd_out = tc.alloc_tile_pool(name="passDout", bufs=4, side="right")
    ps_D = tc.alloc_tile_pool(name="psumD", bufs=4, space="PSUM", side="left")
    if USE_PE_TRANSPOSE_D:
        ps_Dt = tc.alloc_tile_pool(name="psumDt", bufs=4, space="PSUM", side="left")

    for rt in range(n_row_tiles):
        c, t = divmod(rt, tpc)
        # x^T tiles for this row block: xT8[i, g, n] = x[rt*128+n, g*128+i]
        if USE_PE_TRANSPOSE_D:
            xT8 = pd.tile([PART, 8, PART], bf16, tag="xT", name="xT")
            for g in range(8):
                tp = ps_Dt.tile([PART, PART], bf16, tag="xtr", name="xtr")
                nc.tensor.transpose(
                    tp[:], slab[c][:, t, g * PART:(g + 1) * PART], id_bf[:]
                )
                nc.scalar.copy(xT8[:, g, :], tp[:])
        else:
            xT8 = pd.tile([PART, 8, PART], bf16, tag="xT", name="xT")
            nc.scalar.dma_start(xT8[:], slab[c][:, t, :], transpose=True)
        ot = pd_out.tile([PART, D], f32, tag="ot", name="ot")
        for nb in range(2):
            pt = ps_D.tile([PART, BLOCK], f32, tag="outp", name="outp")
            for g in range(8):
                nc.tensor.matmul(
                    pt[:],
                    xT8[:, g, :],
                    wT[:, g, nb * BLOCK:(nb + 1) * BLOCK],
                    start=(g == 0),
                    stop=(g == 7),
                )
            nc.vector.tensor_add(
                ot[:, nb * BLOCK:(nb + 1) * BLOCK], pt[:],
                b_rep[:, nb * BLOCK:(nb + 1) * BLOCK],
            )
        nc.sync.dma_start(out[rt * PART:(rt + 1) * PART, :], ot[:])

    if USE_PE_TRANSPOSE_D:
        ps_Dt.release()
    ps_D.release()
    pd_out.release()
    pd.release()
    wts2.release()
    consts.release()
    persist.release()
    dram.release()


# ---------------------------------------------------------------------------
def make_aux_inputs():
    import ml_dtypes

    eye15 = np.zeros((PART, 4, BLOCK), np.float32)
    for b in range(4):
        for p in range(PART):
            eye15[p, b, b * PART + p] = 1.5
    return {
        "eye15": eye15.astype(ml_dtypes.bfloat16),
        "id_bf16": np.eye(PART, dtype=ml_dtypes.bfloat16),
        "id_f32": np.eye(PART, dtype=np.float32),
    }


_NC_CACHE = {}


def get_nc(n_row_tiles=N_ROW_TILES):
    if n_row_tiles not in _NC_CACHE:
        _NC_CACHE[n_row_tiles] = build_nc(n_row_tiles)
    return _NC_CACHE[n_row_tiles]


def make_in_maps(x, weight, bias, n_row_tiles=N_ROW_TILES):
    aux = make_aux_inputs()
    x = np.ascontiguousarray(np.asarray(x, dtype=np.float32))
    weight = np.ascontiguousarray(np.asarray(weight, dtype=np.float32))
    bias = np.asarray(bias, dtype=np.float32)
    bias_rep = np.ascontiguousarray(np.tile(bias[None, :], (PART, 1)))
    rows_pc = n_row_tiles * PART
    in_maps = []
    for i in range(N_CORES):
        m = {"x": x[i * rows_pc:(i + 1) * rows_pc], "weight": weight,
             "bias_rep": bias_rep}
        m.update(aux)
        in_maps.append(m)
    return in_maps


def kernel(x, weight, bias):
    nc = get_nc()
    in_maps = make_in_maps(x, weight, bias)
    res = bass_utils.run_bass_kernel_spmd(
        nc, in_maps, core_ids=list(range(N_CORES))
    )
    return np.concatenate([r["out"] for r in res.results], axis=0)
